# revision 4
# baseline (speedup 1.0000x reference)
"""Trainium2 Bass kernel for an 8-layer tanh RNN (H=10, D_IN=1) + linear head.

Problem: x [512, 4096, 1] -> 8 stacked tanh-RNN layers (hidden 10) -> linear.
Returns (y [512, 4096, 1], h1 [8, 512, 10]).

Strategy:
- Data-parallel over batch: 512 / 8 cores = 64 batch elements per core.
- Layer-wavefront: at wavefront step s, layer l computes time t = s - l.
  All 8 layers' recurrent+input matvecs collapse into ONE matmul per step:
    rhs  = state slot [81, 64]: rows 0..79 = h[l][j] (row 10l+j),
           row 80 = x_t (DMA-staged ahead of time)
    lhsT = [81, 112]: output col 0 = y head (w_lin against layer-7 rows),
           cols 32+10l+j = pre-activation of unit (l,j):
           W_hh[l] block-diagonal + W_ih[l] sub-diagonal + w_ih0 on the x row
  PSUM out [112, 64]: row 0 = y_t (one step delayed), rows 32..111 = preacts.
  ACT applies tanh(+bias) from psum[32:112] into the next state slot's rows
  0..80. DVE extracts psum[0:1] into a y staging row; DMAs stream x in and
  y out in chunks. Engine APs stay 32-partition aligned (HW requirement);
  all unaligned accesses (x row 80, h0 warmup restores, h1 rows) are DMAs.
- Serial chain is T+L steps of (PE matmul -> ACT tanh) instead of T*L.
"""

import numpy as np

B, T, H, L = 512, 4096, 10, 8
NCORES = 8
BC = B // NCORES          # 64 batch per core
NS = L * H                # 80 state rows
XROW = NS                 # partition holding x_t in the rhs
KDIM = NS + 1             # 81 = matmul contraction size
MDIM = 97                 # matmul output partitions: 0..79 states, 96 = y
YROW = 96                 # psum partition carrying the y head output
NSTEP = T + L             # 4104 matmul steps (s = 0 .. T+7)
NSB = 128                 # state slots (rotation period)
NPS = 4                   # psum buffers
XCH = 64                  # steps per x-staging DMA chunk (NSB / 2)
NXC = T // XCH            # 64 x chunks
YCH = 256                 # y timesteps per output DMA chunk
NYC = T // YCH            # 16 y chunks


def _build_program(b_lin_val: float):
    import concourse.bass as bass
    import concourse.mybir as mybir
    from contextlib import ExitStack

    DT = mybir.dt.float32
    nc = bass.Bass()

    x_d = nc.declare_dram_parameter("x_ck", [NXC, XCH * BC], DT, isOutput=False)
    w_d = nc.declare_dram_parameter("lhsT", [KDIM, MDIM], DT, isOutput=False)
    b_d = nc.declare_dram_parameter("bias", [NS, 1], DT, isOutput=False)
    h0_d = nc.declare_dram_parameter("h0", [NS, BC], DT, isOutput=False)
    y_d = nc.declare_dram_parameter("y_ck", [NYC, YCH * BC], DT, isOutput=True)
    h1_d = nc.declare_dram_parameter("h1", [NS, BC], DT, isOutput=True)

    Tanh = mybir.ActivationFunctionType.Tanh

    with ExitStack() as ctx:
        wT = ctx.enter_context(nc.sbuf_tensor("wTs", [KDIM, MDIM], DT))
        bs = ctx.enter_context(nc.sbuf_tensor("bss", [NS, 1], DT))
        st = ctx.enter_context(nc.sbuf_tensor("sts", [KDIM, NSB * BC], DT))
        yst = ctx.enter_context(nc.sbuf_tensor("yst", [YROW + 1, 2 * YCH * BC], DT))
        ps = [ctx.enter_context(nc.psum_tensor(f"ps{i}", [MDIM, BC], DT))
              for i in range(NPS)]

        dma_in = ctx.enter_context(nc.semaphore("dma_in"))
        xsem = ctx.enter_context(nc.semaphore("xsem"))
        h0r_sem = ctx.enter_context(nc.semaphore("h0r_sem"))
        ydma_sem = ctx.enter_context(nc.semaphore("ydma_sem"))
        h1dma_sem = ctx.enter_context(nc.semaphore("h1dma_sem"))
        pe_sem = ctx.enter_context(nc.semaphore("pe_sem"))
        act_sem = ctx.enter_context(nc.semaphore("act_sem"))
        dve_sem = ctx.enter_context(nc.semaphore("dve_sem"))

        def slot(u, r0=0, r1=KDIM):
            k = (u % NSB) * BC
            return st[r0:r1, k:k + BC]

        block = ctx.enter_context(nc.Block())

        @block.sync
        def _(sync):
            sync.dma_start(out=wT[:, :], in_=w_d[:, :]).then_inc(dma_in, 16)
            sync.dma_start(out=bs[:, :], in_=b_d[:, :]).then_inc(dma_in, 16)
            # initial state S_0 rows 0..79 <- h0
            sync.dma_start(out=slot(0, 0, NS), in_=h0_d[:, :]).then_inc(dma_in, 16)
            # first two x chunks
            for j in range(2):
                sync.dma_start(
                    out=st[XROW:XROW + 1, (j % 2) * XCH * BC:(j % 2 + 1) * XCH * BC],
                    in_=x_d[j, :],
                ).then_inc(xsem, 16)
            # warmup restores: after ACT step s, put h0 back into rows of
            # layers that have not started yet (slot s+1, rows 10(s+1)..80)
            for s in range(L - 1):
                r0 = H * (s + 1)
                sync.wait_ge(act_sem, s + 1)
                sync.dma_start(
                    out=slot(s + 1, r0, NS), in_=h0_d[r0:NS, :]
                ).then_inc(h0r_sem, 16)

            # steady-state DMAs, ordered by the chain step that unblocks them
            events = []
            for j in range(2, NXC):
                def emit_x(sync, j=j):
                    sync.wait_ge(pe_sem, XCH * (j - 1))
                    sync.dma_start(
                        out=st[XROW:XROW + 1,
                               (j % 2) * XCH * BC:(j % 2 + 1) * XCH * BC],
                        in_=x_d[j, :],
                    ).then_inc(xsem, 16)
                events.append((XCH * (j - 1), emit_x))
            for k in range(NYC):
                def emit_y(sync, k=k):
                    sync.wait_ge(dve_sem, YCH * (k + 1))
                    sync.dma_start(
                        out=y_d[k, :],
                        in_=yst[YROW:YROW + 1,
                                (k % 2) * YCH * BC:(k % 2 + 1) * YCH * BC],
                    ).then_inc(ydma_sem, 16)
                events.append((YCH * (k + 1) + 8, emit_y))
            for l in range(L):
                def emit_h1(sync, l=l):
                    sync.wait_ge(act_sem, T + l)
                    sync.dma_start(
                        out=h1_d[H * l:H * (l + 1), :],
                        in_=slot(T + l, H * l, H * (l + 1)),
                    ).then_inc(h1dma_sem, 16)
                events.append((T + l, emit_h1))
            events.sort(key=lambda e: e[0])
            for _, fn in events:
                fn(sync)
            sync.wait_ge(ydma_sem, 16 * NYC)
            sync.wait_ge(h1dma_sem, 16 * L)

        @block.tensor
        def _(tensor):
            for u in range(NSTEP):
                if u == 0:
                    tensor.wait_ge(dma_in, 48)
                else:
                    tensor.wait_ge(act_sem, u)
                if 1 <= u <= L - 1:
                    tensor.wait_ge(h0r_sem, 16 * u)
                if u % XCH == 0 and u // XCH < NXC:
                    tensor.wait_ge(xsem, 16 * (u // XCH + 1))
                if u >= L + NPS:
                    tensor.wait_ge(dve_sem, u - L - NPS + 1)
                nc.tensor.matmul(
                    out=ps[u % NPS][:, :],
                    lhsT=wT[:, :],
                    rhs=slot(u),
                    start=True,
                    stop=True,
                ).then_inc(pe_sem, 1)

        @block.scalar
        def _(scalar):
            for s in range(NSTEP - 1):  # 0 .. 4102
                scalar.wait_ge(pe_sem, s + 1)
                nc.scalar.activation(
                    out=slot(s + 1, 0, NS),
                    in_=ps[s % NPS][0:NS, :],
                    func=Tanh,
                    bias=bs[:, 0:1],
                    scale=1.0,
                ).then_inc(act_sem, 1)

        @block.vector
        def _(vector):
            for s in range(L, NSTEP):  # y_t for t = s - L
                t = s - L
                if t % YCH == 0 and t // YCH >= 2:
                    vector.wait_ge(ydma_sem, 16 * (t // YCH - 1))
                vector.wait_ge(pe_sem, s + 1)
                off = (t % (2 * YCH)) * BC
                nc.vector.tensor_scalar_add(
                    yst[YROW:YROW + 1, off:off + BC],
                    ps[s % NPS][YROW:YROW + 1, :],
                    b_lin_val,
                ).then_inc(dve_sem, 1)

    return nc


def _prep_inputs(x, h0, w_ih0, w_ih_rest, w_hh, b_ih, b_hh, w_lin):
    """Host-side packing: combined block lhsT, bias, per-core x/h0 shards."""
    lhsT = np.zeros((KDIM, MDIM), np.float32)
    for l in range(L):
        cols = slice(H * l, H * (l + 1))
        lhsT[H * l:H * (l + 1), cols] = w_hh[l].T
        if l >= 1:
            lhsT[H * (l - 1):H * l, cols] = w_ih_rest[l - 1].T
    lhsT[XROW, 0:H] = w_ih0[:, 0]
    lhsT[H * (L - 1):NS, YROW] = w_lin[0, :]

    bias = (b_ih + b_hh).reshape(NS, 1).astype(np.float32)

    in_maps = []
    for c in range(NCORES):
        bsl = slice(c * BC, (c + 1) * BC)
        # x chunks: x_ck[j, jj*64 + b] = x[b, j*64 + jj]
        xc = np.ascontiguousarray(x[bsl, :, 0]).T  # [T, BC]
        xc = np.ascontiguousarray(xc.reshape(NXC, XCH * BC))
        # h0 shard [8, 64, 10] -> [80, 64] with row 10l+j = h0[l, :, j]
        h0c = np.ascontiguousarray(h0[:, bsl, :]).transpose(0, 2, 1).reshape(NS, BC)
        in_maps.append({
            "x_ck": xc.astype(np.float32),
            "lhsT": lhsT,
            "bias": bias,
            "h0": np.ascontiguousarray(h0c, dtype=np.float32),
        })
    return in_maps


def kernel(x, h0, w_ih0, w_ih_rest, w_hh, b_ih, b_hh, w_lin, b_lin, _trace=False):
    from concourse.bass_utils import run_bass_kernel_spmd

    nc = _build_program(float(np.asarray(b_lin).reshape(-1)[0]))
    in_maps = _prep_inputs(x, h0, w_ih0, w_ih_rest, w_hh, b_ih, b_hh, w_lin)
    res = run_bass_kernel_spmd(nc, in_maps, list(range(NCORES)), trace=_trace)

    y = np.empty((B, T, 1), np.float32)
    h1 = np.empty((L, B, H), np.float32)
    for c in range(NCORES):
        bsl = slice(c * BC, (c + 1) * BC)
        yc = res.results[c]["y_ck"].reshape(T, BC)  # [t, b]
        y[bsl, :, 0] = yc.T
        h1c = res.results[c]["h1"]  # [80, 64]
        h1[:, bsl, :] = h1c.reshape(L, H, BC).transpose(0, 2, 1)
    if _trace:
        kernel._last_exec_time_ns = res.exec_time_ns
    return y, h1


# revision 7
# speedup vs baseline: 47.2373x; 47.2373x over previous
"""Trainium2 Bass kernel for an 8-layer tanh RNN (H=10, D_IN=1) + linear head.

Problem: x [512, 4096, 1] -> 8 stacked tanh-RNN layers (hidden 10) -> linear.
Returns (y [512, 4096, 1], h1 [8, 512, 10]).

Strategy:
- Data-parallel over batch: 512 / 8 cores = 64 batch elements per core.
- Layer-wavefront: at wavefront step s, layer l computes time t = s - l.
  All 8 layers' recurrent+input matvecs collapse into ONE matmul per step:
    rhs  = state slot [81, 64]: rows 0..79 = h[l][j] (row 10l+j),
           row 80 = x_t (DMA-staged ahead of time)
    lhsT = [81, 97]: col 10l+j = pre-activation of unit (l,j)
           (W_hh[l] block-diagonal + W_ih[l] sub-diagonal + w_ih0 on the
           x row); col 96 = y head (w_lin against layer-7 rows).
  PSUM out [97, 64]: rows 0..79 = pre-activations, row 96 = y_t (computed
  one step delayed from the layer-7 state).
  ACT applies tanh(+bias) from psum[0:80] into the next state slot's rows
  0..79. DVE extracts psum[96:97] into a y staging row at partition 96;
  DMAs stream x in and y out in chunks. Engine APs keep aligned partition
  bases (HW requirement: base/range must fit a 32/64/128-aligned window);
  all unaligned accesses (x row 80, h0 warmup restores, h1 rows) are DMAs,
  which have no partition alignment constraint.
- Serial chain is T+L steps of (PE matmul -> ACT tanh) instead of T*L;
  wall time is dominated by 4104 x (PE latency + tanh latency + 2 sem hops).
"""

import numpy as np

B, T, H, L = 512, 4096, 10, 8
NCORES = 8
BC = B // NCORES          # 64 batch per core
NS = L * H                # 80 state rows
XROW = NS                 # partition holding x_t in the rhs
KDIM = NS + 1             # 81 = matmul contraction size
MDIM = 97                 # matmul output partitions: 0..79 states, 96 = y
YROW = 96                 # psum partition carrying the y head output
NSTEP = T + L             # 4104 matmul steps (s = 0 .. T+7)
NSB = 128                 # state slots (rotation period)
NPS = 4                   # psum buffers
XCH = 64                  # steps per x-staging DMA chunk (NSB / 2)
NXC = T // XCH            # 64 x chunks
YCH = 256                 # y timesteps per output DMA chunk
NYC = T // YCH            # 16 y chunks


def _build_program(b_lin_val: float):
    import concourse.bass as bass
    import concourse.mybir as mybir
    from contextlib import ExitStack

    DT = mybir.dt.float32
    nc = bass.Bass()

    x_d = nc.declare_dram_parameter("x_ck", [NXC, XCH * BC], DT, isOutput=False)
    w_d = nc.declare_dram_parameter("lhsT", [KDIM, MDIM], DT, isOutput=False)
    b_d = nc.declare_dram_parameter("bias", [NS, 1], DT, isOutput=False)
    h0_d = nc.declare_dram_parameter("h0", [NS, BC], DT, isOutput=False)
    y_d = nc.declare_dram_parameter("y_ck", [NYC, YCH * BC], DT, isOutput=True)
    h1_d = nc.declare_dram_parameter("h1", [NS, BC], DT, isOutput=True)

    Tanh = mybir.ActivationFunctionType.Tanh

    with ExitStack() as ctx:
        wT = ctx.enter_context(nc.sbuf_tensor("wTs", [KDIM, MDIM], DT))
        bs = ctx.enter_context(nc.sbuf_tensor("bss", [NS, 1], DT))
        st = ctx.enter_context(nc.sbuf_tensor("sts", [KDIM, NSB * BC], DT))
        yst = ctx.enter_context(nc.sbuf_tensor("yst", [YROW + 1, 2 * YCH * BC], DT))
        ps = [ctx.enter_context(nc.psum_tensor(f"ps{i}", [MDIM, BC], DT))
              for i in range(NPS)]

        dma_in = ctx.enter_context(nc.semaphore("dma_in"))
        # x / y stream DMAs alternate between two semaphores (chunk parity):
        # increments of independent DMAs on one semaphore can land out of
        # order, so a shared counter would let chunk j+1 satisfy the wait
        # meant for chunk j. Within a parity, issue order is enforced by the
        # pe/dve gating, so per-parity counters are race-free.
        xsems = [ctx.enter_context(nc.semaphore(f"xsem{p}")) for p in range(2)]
        h0r_sem = ctx.enter_context(nc.semaphore("h0r_sem"))
        ydsems = [ctx.enter_context(nc.semaphore(f"ydma{p}")) for p in range(2)]
        h1dma_sem = ctx.enter_context(nc.semaphore("h1dma_sem"))
        pe_sem = ctx.enter_context(nc.semaphore("pe_sem"))
        act_sem = ctx.enter_context(nc.semaphore("act_sem"))
        dve_sem = ctx.enter_context(nc.semaphore("dve_sem"))

        def slot(u, r0=0, r1=KDIM):
            k = (u % NSB) * BC
            return st[r0:r1, k:k + BC]

        block = ctx.enter_context(nc.Block())

        @block.sync
        def _(sync):
            sync.dma_start(out=wT[:, :], in_=w_d[:, :]).then_inc(dma_in, 16)
            sync.dma_start(out=bs[:, :], in_=b_d[:, :]).then_inc(dma_in, 16)
            # initial state S_0 rows 0..79 <- h0
            sync.dma_start(out=slot(0, 0, NS), in_=h0_d[:, :]).then_inc(dma_in, 16)
            # first two x chunks
            for j in range(2):
                sync.dma_start(
                    out=st[XROW:XROW + 1, (j % 2) * XCH * BC:(j % 2 + 1) * XCH * BC],
                    in_=x_d[j, :],
                ).then_inc(xsems[j % 2], 16)
            # warmup restores: after ACT step s, put h0 back into rows of
            # layers that have not started yet (slot s+1, rows 10(s+1)..80)
            for s in range(L - 1):
                r0 = H * (s + 1)
                sync.wait_ge(act_sem, s + 1)
                sync.dma_start(
                    out=slot(s + 1, r0, NS), in_=h0_d[r0:NS, :]
                ).then_inc(h0r_sem, 16)

            # steady-state DMAs, ordered by the chain step that unblocks them
            events = []
            for j in range(2, NXC):
                def emit_x(sync, j=j):
                    sync.wait_ge(pe_sem, XCH * (j - 1))
                    sync.dma_start(
                        out=st[XROW:XROW + 1,
                               (j % 2) * XCH * BC:(j % 2 + 1) * XCH * BC],
                        in_=x_d[j, :],
                    ).then_inc(xsems[j % 2], 16)
                events.append((XCH * (j - 1), emit_x))
            for k in range(NYC):
                def emit_y(sync, k=k):
                    sync.wait_ge(dve_sem, YCH * (k + 1))
                    sync.dma_start(
                        out=y_d[k, :],
                        in_=yst[YROW:YROW + 1,
                                (k % 2) * YCH * BC:(k % 2 + 1) * YCH * BC],
                    ).then_inc(ydsems[k % 2], 16)
                events.append((YCH * (k + 1) + 8, emit_y))
            for l in range(L):
                def emit_h1(sync, l=l):
                    sync.wait_ge(act_sem, T + l)
                    sync.dma_start(
                        out=h1_d[H * l:H * (l + 1), :],
                        in_=slot(T + l, H * l, H * (l + 1)),
                    ).then_inc(h1dma_sem, 16)
                events.append((T + l, emit_h1))
            events.sort(key=lambda e: e[0])
            for _, fn in events:
                fn(sync)
            sync.wait_ge(ydsems[0], 16 * (NYC // 2))
            sync.wait_ge(ydsems[1], 16 * (NYC // 2))
            sync.wait_ge(h1dma_sem, 16 * L)

        @block.tensor
        def _(tensor):
            for u in range(NSTEP):
                if u == 0:
                    tensor.wait_ge(dma_in, 48)
                else:
                    tensor.wait_ge(act_sem, u)
                if 1 <= u <= L - 1:
                    tensor.wait_ge(h0r_sem, 16 * u)
                if u % XCH == 0 and u // XCH < NXC:
                    j = u // XCH
                    tensor.wait_ge(xsems[j % 2], 16 * (j // 2 + 1))
                if u >= L + NPS:
                    tensor.wait_ge(dve_sem, u - L - NPS + 1)
                nc.tensor.matmul(
                    out=ps[u % NPS][:, :],
                    lhsT=wT[:, :],
                    rhs=slot(u),
                    start=True,
                    stop=True,
                ).then_inc(pe_sem, 1)

        @block.scalar
        def _(scalar):
            for s in range(NSTEP - 1):  # 0 .. 4102
                scalar.wait_ge(pe_sem, s + 1)
                nc.scalar.activation(
                    out=slot(s + 1, 0, NS),
                    in_=ps[s % NPS][0:NS, :],
                    func=Tanh,
                    bias=bs[:, 0:1],
                    scale=1.0,
                ).then_inc(act_sem, 1)

        @block.vector
        def _(vector):
            for s in range(L, NSTEP):  # y_t for t = s - L
                t = s - L
                if t % YCH == 0 and t // YCH >= 2:
                    kk = t // YCH
                    vector.wait_ge(ydsems[kk % 2], 16 * ((kk - 2) // 2 + 1))
                vector.wait_ge(pe_sem, s + 1)
                off = (t % (2 * YCH)) * BC
                nc.vector.tensor_scalar_add(
                    yst[YROW:YROW + 1, off:off + BC],
                    ps[s % NPS][YROW:YROW + 1, :],
                    b_lin_val,
                ).then_inc(dve_sem, 1)

    return nc


def _prep_inputs(x, h0, w_ih0, w_ih_rest, w_hh, b_ih, b_hh, w_lin):
    """Host-side packing: combined block lhsT, bias, per-core x/h0 shards."""
    lhsT = np.zeros((KDIM, MDIM), np.float32)
    for l in range(L):
        cols = slice(H * l, H * (l + 1))
        lhsT[H * l:H * (l + 1), cols] = w_hh[l].T
        if l >= 1:
            lhsT[H * (l - 1):H * l, cols] = w_ih_rest[l - 1].T
    lhsT[XROW, 0:H] = w_ih0[:, 0]
    lhsT[H * (L - 1):NS, YROW] = w_lin[0, :]

    bias = (b_ih + b_hh).reshape(NS, 1).astype(np.float32)

    in_maps = []
    for c in range(NCORES):
        bsl = slice(c * BC, (c + 1) * BC)
        # x chunks: x_ck[j, jj*64 + b] = x[b, j*64 + jj]
        xc = np.ascontiguousarray(x[bsl, :, 0]).T  # [T, BC]
        xc = np.ascontiguousarray(xc.reshape(NXC, XCH * BC))
        # h0 shard [8, 64, 10] -> [80, 64] with row 10l+j = h0[l, :, j]
        h0c = np.ascontiguousarray(h0[:, bsl, :]).transpose(0, 2, 1).reshape(NS, BC)
        in_maps.append({
            "x_ck": xc.astype(np.float32),
            "lhsT": lhsT,
            "bias": bias,
            "h0": np.ascontiguousarray(h0c, dtype=np.float32),
        })
    return in_maps


def kernel(x, h0, w_ih0, w_ih_rest, w_hh, b_ih, b_hh, w_lin, b_lin, _trace=False):
    from concourse.bass_utils import run_bass_kernel_spmd

    nc = _build_program(float(np.asarray(b_lin).reshape(-1)[0]))
    in_maps = _prep_inputs(x, h0, w_ih0, w_ih_rest, w_hh, b_ih, b_hh, w_lin)
    kernel._last_nc = nc
    kernel._last_in_maps = in_maps
    res = run_bass_kernel_spmd(nc, in_maps, list(range(NCORES)), trace=_trace)

    y = np.empty((B, T, 1), np.float32)
    h1 = np.empty((L, B, H), np.float32)
    for c in range(NCORES):
        bsl = slice(c * BC, (c + 1) * BC)
        yc = res.results[c]["y_ck"].reshape(T, BC)  # [t, b]
        y[bsl, :, 0] = yc.T
        h1c = res.results[c]["h1"]  # [80, 64]
        h1[:, bsl, :] = h1c.reshape(L, H, BC).transpose(0, 2, 1)
    if _trace:
        kernel._last_exec_time_ns = res.exec_time_ns
    return y, h1
